# revision 11
# baseline (speedup 1.0000x reference)
"""TRN2 Bass kernel for nn_MultiPrecisionLinear (moe_routing).

Reference computation:
    xs = x.reshape(P, bpp, S, Din)            # P=8 paths
    W  = weight_bank[assigned_bits]           # [P, Dout, Din]
    out = einsum('pbsi,poi->pbso', xs, W) + bias

Sharding: path-parallel. Core p holds path p's batch slice
[bpp*S, Din] = [32768, 256], its selected weight (as [Din, Dout]) and the
bias. All layout work happens on host so the device kernel is a pure
streaming matmul over fp32r:

  x is pre-transposed AND pre-chunked on host into contiguous 1MB blocks
  xt[c] = [128(i%128), 2(i//128), MC(m)]  -> each DMA reads one contiguous
  block, 8KB contiguous per partition (minimal descriptor count).

  per chunk c:
    DMA in  xt[c] (1MB, Sync HWDGE)
    8 fp32r matmuls (2 oc x 2 ic x 2 halves, N=512) -> out_T in PSUM
    bias add fused with PSUM->SBUF move (ACT Identity for oc=0, DVE
    tensor_scalar_add for oc=1; bias is per-partition in this layout)
    DMA out [128, 2, MC] (1MB, Scalar HWDGE) -> out6[c]

fp32r: full-rate PE (1 cyc/row) at ~1.5e-4 rel RMS error (HW-measured;
fp32 is 4x slower, bf16 is 16x less accurate). DRAM inputs are declared
float32r with raw f32 bytes — HW rounds internally (verified equivalent
to explicit on-device rounding).
"""

import numpy as np

import concourse.bacc as bacc
import concourse.mybir as mybir
import concourse.tile as tile

F32 = mybir.dt.float32
F32R = mybir.dt.float32r
AF = mybir.ActivationFunctionType

# Problem geometry (hardcoded per spec).
P = 8          # paths == cores
BPP = 8        # batch per path
S = 4096
DIN = 256
DOUT = 256
M = BPP * S    # rows per core = 32768
MC = 2048      # m-columns per chunk (2MB DMA blocks)

_CACHE = {}


def chunk_plan(m=M, mc=MC):
    """Column count per chunk. Small leading chunks spin up the
    compute/write pipeline while the first big reads stream in."""
    lead = [512] * 4
    rest = (m - sum(lead)) // mc
    plan = lead + [mc] * rest
    assert sum(plan) == m
    return plan


def build_nc(m=M, mc=MC):
    key = (m, mc)
    if key in _CACHE:
        return _CACHE[key]

    plan = chunk_plan(m, mc)

    nc = bacc.Bacc("TRN2", target_bir_lowering=False, debug=False)
    # xt is a flat [128, 2, m]-per-chunk sequence of contiguous blocks
    xt_d = nc.dram_tensor("xt", [128 * 2 * m], F32R, kind="ExternalInput")
    w_d = nc.dram_tensor("w", [2, 128, DOUT], F32R, kind="ExternalInput")
    bias_d = nc.dram_tensor("bias2", [2, 128], F32, kind="ExternalInput")
    out_d = nc.dram_tensor("out6", [128 * 2 * m], F32, kind="ExternalOutput")

    with tile.TileContext(nc) as tc:
        with (
            tc.tile_pool(name="const", bufs=1) as const,
            tc.tile_pool(name="xin", bufs=4) as xin_pool,
            tc.tile_pool(name="oout", bufs=3) as oout_pool,
            tc.tile_pool(name="psum", bufs=2, space="PSUM") as psum,
        ):
            # setup DMAs on the Scalar HWDGE ring (idle early; Sync leads
            # with chunk 0, and HWDGE beats SWDGE's slow Q7 spin-up)
            w_sb = const.tile([128, 2, DOUT], F32R, tag="w_sb")
            nc.scalar.dma_start(w_sb[:], w_d[:].rearrange("c p n -> p c n"))
            bias_sb = const.tile([128, 2], F32, tag="bias_sb")
            nc.scalar.dma_start(bias_sb[:], bias_d[:].rearrange("c p -> p c"))

            off = 0
            for c, cw in enumerate(plan):
                nh = cw // 512
                blk_in = xt_d[off : off + 128 * 2 * cw].rearrange(
                    "(p c m) -> p c m", p=128, c=2
                )
                blk_out = out_d[off : off + 128 * 2 * cw].rearrange(
                    "(p c m) -> p c m", p=128, c=2
                )
                off += 128 * 2 * cw
                xt = xin_pool.tile([128, 2, cw], F32R, name=f"xt{c}", tag="xt")
                nc.sync.dma_start(xt[:], blk_in)
                osb = oout_pool.tile([128, 2, cw], F32, name=f"osb{c}", tag="osb")
                for oc in range(2):
                    for h in range(nh):
                        po = psum.tile(
                            [128, 512], F32, name=f"po{oc}{h}", tag=f"po{oc}{h % 2}"
                        )
                        for ic in range(2):
                            nc.tensor.matmul(
                                po[:],
                                w_sb[:, ic, oc * 128 : (oc + 1) * 128],
                                xt[:, ic, h * 512 : (h + 1) * 512],
                                start=(ic == 0),
                                stop=(ic == 1),
                            )
                        dst = osb[:, oc, h * 512 : (h + 1) * 512]
                        if oc == 0:
                            nc.scalar.activation(
                                dst, po[:], AF.Identity,
                                bias=bias_sb[:, oc : oc + 1],
                            )
                        else:
                            nc.vector.tensor_scalar_add(
                                dst, po[:], bias_sb[:, oc : oc + 1]
                            )
                nc.scalar.dma_start(blk_out, osb[:])
    nc.compile()
    _CACHE[key] = nc
    return nc


def make_in_maps(x, weight_bank, bias, assigned_bits, m=M, mc=MC):
    """Host-side sharding + layout: per-core input dicts."""
    x = np.asarray(x, dtype=np.float32)
    weight_bank = np.asarray(weight_bank, dtype=np.float32)
    bias = np.asarray(bias, dtype=np.float32)
    idx = np.asarray(assigned_bits).astype(np.int64)

    plan = chunk_plan(m, mc)
    bias2 = np.ascontiguousarray(bias.reshape(2, 128))
    xs = x.reshape(P, m, DIN)
    in_maps = []
    for p in range(P):
        # per chunk block[q, ic, j] = x_p[m0 + j, ic*128 + q]
        parts = []
        m0 = 0
        for cw in plan:
            parts.append(
                xs[p][m0 : m0 + cw]
                .reshape(cw, 2, 128)
                .transpose(2, 1, 0)
                .ravel()
            )
            m0 += cw
        xt = np.concatenate(parts)
        w_io = np.ascontiguousarray(weight_bank[idx[p]].T)  # [Din, Dout]
        in_maps.append(
            {
                "xt": xt,
                "w": w_io.reshape(2, 128, DOUT),
                "bias2": bias2,
            }
        )
    return in_maps


def assemble_out(results, m=M, mc=MC):
    plan = chunk_plan(m, mc)
    outs = []
    for r in results:
        flat = r["out6"]
        rows = []
        off = 0
        for cw in plan:
            blk = flat[off : off + 128 * 2 * cw].reshape(128, 2, cw)
            rows.append(blk.transpose(2, 1, 0).reshape(cw, DOUT))
            off += 128 * 2 * cw
        outs.append(np.concatenate(rows, axis=0))
    out = np.stack(outs)  # [P, m, DOUT]
    return np.ascontiguousarray(out.reshape(P * BPP, S, DOUT))


def kernel(x, weight_bank, bias, assigned_bits):
    from concourse.bass_utils import run_bass_kernel_spmd

    nc = build_nc()
    in_maps = make_in_maps(x, weight_bank, bias, assigned_bits)
    res = run_bass_kernel_spmd(nc, in_maps, core_ids=list(range(P)))
    return assemble_out(res.results)


# revision 19
# speedup vs baseline: 1.1417x; 1.1417x over previous
"""TRN2 Bass kernel for nn_MultiPrecisionLinear (moe_routing).

Reference computation:
    xs = x.reshape(P, bpp, S, Din)            # P=8 paths
    W  = weight_bank[assigned_bits]           # [P, Dout, Din]
    out = einsum('pbsi,poi->pbso', xs, W) + bias

Sharding: path-parallel. Core p holds path p's batch slice
[bpp*S, Din] = [32768, 256], its selected weight (as [Din, Dout]) and the
bias. All layout work happens on host so the device kernel is a pure
streaming matmul over fp32r:

  x is pre-transposed AND pre-chunked on host into contiguous 1MB blocks
  xt[c] = [128(i%128), 2(i//128), MC(m)]  -> each DMA reads one contiguous
  block, 8KB contiguous per partition (minimal descriptor count).

  per chunk c:
    DMA in  xt[c] (1MB, Sync HWDGE)
    8 fp32r matmuls (2 oc x 2 ic x 2 halves, N=512) -> out_T in PSUM
    bias add fused with PSUM->SBUF move (ACT Identity for oc=0, DVE
    tensor_scalar_add for oc=1; bias is per-partition in this layout)
    DMA out [128, 2, MC] (1MB, Scalar HWDGE) -> out6[c]

fp32r: full-rate PE (1 cyc/row) at ~1.5e-4 rel RMS error (HW-measured;
fp32 is 4x slower, bf16 is 16x less accurate). DRAM inputs are declared
float32r with raw f32 bytes — HW rounds internally (verified equivalent
to explicit on-device rounding).
"""

import numpy as np

import concourse.bacc as bacc
import concourse.mybir as mybir
import concourse.tile as tile

F32 = mybir.dt.float32
F32R = mybir.dt.float32r
AF = mybir.ActivationFunctionType

# Problem geometry (hardcoded per spec).
P = 8          # paths == cores
BPP = 8        # batch per path
S = 4096
DIN = 256
DOUT = 256
M = BPP * S    # rows per core = 32768
MC = 2048      # m-columns per chunk (2MB DMA blocks)

_CACHE = {}


def chunk_plan(m=M, mc=MC, lead=True, tail=False):
    """Column count per chunk. Small leading chunks spin up the
    compute/write pipeline while the first big reads stream in; small
    trailing chunks tighten the final write drain."""
    lead_part = [512] * 4 if lead else []
    tail_part = [512] * 4 if tail else []
    body = m - sum(lead_part) - sum(tail_part)
    adapter = [body % mc] if body % mc else []
    plan = lead_part + adapter + [mc] * (body // mc) + tail_part
    assert sum(plan) == m and all(cw % 512 == 0 for cw in plan)
    return plan


def build_nc(m=M, mc=MC, lead=True, tail=False):
    key = (m, mc, lead, tail)
    if key in _CACHE:
        return _CACHE[key]

    plan = chunk_plan(m, mc, lead, tail)

    nc = bacc.Bacc("TRN2", target_bir_lowering=False, debug=False)
    # xt is a flat [128, 2, m]-per-chunk sequence of contiguous blocks
    xt_d = nc.dram_tensor("xt", [128 * 2 * m], F32R, kind="ExternalInput")
    w_d = nc.dram_tensor("w", [2, 128, DOUT], F32R, kind="ExternalInput")
    bias_d = nc.dram_tensor("bias2", [2, 128], F32, kind="ExternalInput")
    out_d = nc.dram_tensor("out6", [128 * 2 * m], F32, kind="ExternalOutput")

    bufs_in = 4 if mc <= 2048 else 3
    bufs_out = 3 if mc <= 2048 else 2
    with tile.TileContext(nc) as tc:
        with (
            tc.tile_pool(name="const", bufs=1) as const,
            tc.tile_pool(name="xin", bufs=bufs_in) as xin_pool,
            tc.tile_pool(name="oout", bufs=bufs_out) as oout_pool,
            tc.tile_pool(name="psum", bufs=2, space="PSUM") as psum,
        ):
            # setup DMAs on the Scalar HWDGE ring (idle early; Sync leads
            # with chunk 0, and HWDGE beats SWDGE's slow Q7 spin-up)
            w_sb = const.tile([128, 2, DOUT], F32R, tag="w_sb")
            nc.scalar.dma_start(w_sb[:], w_d[:].rearrange("c p n -> p c n"))
            bias_sb = const.tile([128, 2], F32, tag="bias_sb")
            nc.scalar.dma_start(bias_sb[:], bias_d[:].rearrange("c p -> p c"))

            off = 0
            for c, cw in enumerate(plan):
                nh = cw // 512
                blk_in = xt_d[off : off + 128 * 2 * cw].rearrange(
                    "(p c m) -> p c m", p=128, c=2
                )
                blk_out = out_d[off : off + 128 * 2 * cw].rearrange(
                    "(p c m) -> p c m", p=128, c=2
                )
                off += 128 * 2 * cw
                xt = xin_pool.tile([128, 2, cw], F32R, name=f"xt{c}", tag="xt")
                nc.sync.dma_start(xt[:], blk_in)
                osb = oout_pool.tile([128, 2, cw], F32, name=f"osb{c}", tag="osb")
                for oc in range(2):
                    for h in range(nh):
                        po = psum.tile(
                            [128, 512], F32, name=f"po{oc}{h}", tag=f"po{oc}{h % 2}"
                        )
                        for ic in range(2):
                            nc.tensor.matmul(
                                po[:],
                                w_sb[:, ic, oc * 128 : (oc + 1) * 128],
                                xt[:, ic, h * 512 : (h + 1) * 512],
                                start=(ic == 0),
                                stop=(ic == 1),
                            )
                        dst = osb[:, oc, h * 512 : (h + 1) * 512]
                        if oc == 0:
                            nc.scalar.activation(
                                dst, po[:], AF.Identity,
                                bias=bias_sb[:, oc : oc + 1],
                            )
                        else:
                            nc.vector.tensor_scalar_add(
                                dst, po[:], bias_sb[:, oc : oc + 1]
                            )
                nc.scalar.dma_start(blk_out, osb[:])
    nc.compile()
    _CACHE[key] = nc
    return nc


def make_in_maps(x, weight_bank, bias, assigned_bits, m=M, mc=MC, lead=True, tail=False):
    """Host-side sharding + layout: per-core input dicts."""
    x = np.asarray(x, dtype=np.float32)
    weight_bank = np.asarray(weight_bank, dtype=np.float32)
    bias = np.asarray(bias, dtype=np.float32)
    idx = np.asarray(assigned_bits).astype(np.int64)

    plan = chunk_plan(m, mc, lead, tail)
    bias2 = np.ascontiguousarray(bias.reshape(2, 128))
    xs = x.reshape(P, m, DIN)
    in_maps = []
    for p in range(P):
        # per chunk block[q, ic, j] = x_p[m0 + j, ic*128 + q]
        xt = np.empty(128 * 2 * m, dtype=np.float32)
        m0 = 0
        off = 0
        for cw in plan:
            blk = xt[off : off + 128 * 2 * cw].reshape(128, 2, cw)
            blk[:] = xs[p][m0 : m0 + cw].reshape(cw, 2, 128).transpose(2, 1, 0)
            m0 += cw
            off += 128 * 2 * cw
        w_io = np.ascontiguousarray(weight_bank[idx[p]].T)  # [Din, Dout]
        in_maps.append(
            {
                "xt": xt,
                "w": w_io.reshape(2, 128, DOUT),
                "bias2": bias2,
            }
        )
    return in_maps


def assemble_out(results, m=M, mc=MC, lead=True, tail=False):
    plan = chunk_plan(m, mc, lead, tail)
    out = np.empty((P, m, DOUT), dtype=np.float32)
    for p, r in enumerate(results):
        flat = np.asarray(r["out6"])
        m0 = 0
        off = 0
        for cw in plan:
            blk = flat[off : off + 128 * 2 * cw].reshape(128, 2, cw)
            out[p, m0 : m0 + cw] = blk.transpose(2, 1, 0).reshape(cw, DOUT)
            m0 += cw
            off += 128 * 2 * cw
    return out.reshape(P * BPP, S, DOUT)


def kernel(x, weight_bank, bias, assigned_bits):
    from concourse.bass_utils import run_bass_kernel_spmd

    nc = build_nc()
    in_maps = make_in_maps(x, weight_bank, bias, assigned_bits)
    res = run_bass_kernel_spmd(nc, in_maps, core_ids=list(range(P)))
    return assemble_out(res.results)
